# revision 10
# baseline (speedup 1.0000x reference)
"""Bass/Trainium2 kernel for a single LSTM-cell step + tiny MLP head.

Reference computation (all fp32):
    gates = W_ih @ x + b_ih + W_hh @ h0 + b_hh        # [4H], gate order i,f,g,o
    i, f, g, o = sigmoid/sigmoid/tanh/sigmoid splits
    c = f * c0 + i * g ; h = o * tanh(c)              # [H]
    z = relu(W1 @ h + b1)                             # [32]
    out = sigmoid(W2 @ z + b2)                        # [130]

Sharding (8 NeuronCores, tensor-parallel over the hidden dim):
    Core k owns hidden slice s_k = [k*512, (k+1)*512). It gets the four
    512-row blocks of [W_ih | W_hh | b] for its slice, concatenated into
    one [2048, 12293] matrix (bias folded in via a constant-1 appended to
    the x/h0 vector), padded to K=12416 = 97*128 and stored transposed +
    K-tiled so each SBUF tile is a [128(K), 2048(out)] matmul moving
    operand. The big matvec runs on TensorE accumulating into PSUM.
    The LSTM epilogue + partial MLP dot (W1[:, s_k] @ h_k -> [32]) run
    locally; one tiny AllReduce(32 floats) combines the partials and
    every core finishes the replicated MLP head.
"""

import numpy as np
import ml_dtypes

D = 8196
H = 4096
HS = 512           # hidden slice per core
R = 4 * HS         # gate rows per core (2048)
HID = 32
OUT = 130
NCORES = 8
KD = D + H + 1     # x ++ h0 ++ 1.0 (bias column)
KT = 97            # K tiles of 128 (12416 = 97*128 >= 12293)
KP = KT * 128
NB = 4             # PSUM n-blocks of 512 covering R=2048
G = 4              # K-tiles per weight DMA (2 MiB bf16 per dma_start)

WNP = ml_dtypes.bfloat16   # storage dtype for the big weights + x

_cached = {}

# debug bisection: "h" = stop after LSTM h, "z" = stop after local z_part,
# "full" = everything (default)
import os
STAGE = os.environ.get("KERNEL_STAGE", "full")


def _mybir_wdt(mybir):
    return {
        "bfloat16": mybir.dt.bfloat16,
        "float32": mybir.dt.float32,
        "float8_e4m3fn": mybir.dt.float8e4,
    }[np.dtype(WNP).name]


def build_nc():
    """Build + compile the per-core Bass program (same program on all cores)."""
    import concourse.bass as bass
    import concourse.tile as tile
    from concourse import bacc, mybir

    fp32 = mybir.dt.float32
    wdt = _mybir_wdt(mybir)
    AF = mybir.ActivationFunctionType

    nc = bacc.Bacc("TRN2", target_bir_lowering=False, debug=False,
                   num_devices=NCORES)

    wt_d = nc.dram_tensor("wt", [128, KT * R], wdt, kind="ExternalInput")
    xt_d = nc.dram_tensor("xt", [128, KT], wdt, kind="ExternalInput")
    c0_d = nc.dram_tensor("c0s", [HS], fp32, kind="ExternalInput")
    w1_d = nc.dram_tensor("w1t", [128, (HS // 128) * HID], fp32,
                          kind="ExternalInput")
    b1_d = nc.dram_tensor("b1", [HID], fp32, kind="ExternalInput")
    w2_d = nc.dram_tensor("w2t", [HID, OUT], fp32, kind="ExternalInput")
    b2_d = nc.dram_tensor("b2", [OUT], fp32, kind="ExternalInput")
    out_d = nc.dram_tensor("out", [OUT], fp32, kind="ExternalOutput")

    h_d = nc.dram_tensor("hscratch", [HS], fp32)
    zp_d = nc.dram_tensor("zpart", [HID], fp32)
    zr_d = nc.dram_tensor("zred", [HID], fp32, addr_space="Shared")

    with tile.TileContext(nc) as tc:
        with (
            tc.tile_pool(name="weights", bufs=3) as wpool,
            tc.tile_pool(name="small", bufs=1) as small,
            tc.tile_pool(name="psum", bufs=1, space="PSUM") as psum,
        ):
            # small persistent operands (issued up front, overlap the stream)
            xt_sb = small.tile([128, KT], wdt)
            nc.gpsimd.dma_start(xt_sb[:], xt_d[:])
            c0_sb = small.tile([1, HS], fp32)
            nc.gpsimd.dma_start(c0_sb[:], c0_d[None, :])
            w1_sb = small.tile([128, HS // 128, HID], fp32)
            nc.gpsimd.dma_start(w1_sb[:], w1_d[:])
            b1_sb = small.tile([HID, 1], fp32)
            nc.gpsimd.dma_start(b1_sb[:], b1_d[:, None])
            w2_sb = small.tile([HID, OUT], fp32)
            nc.gpsimd.dma_start(w2_sb[:], w2_d[:])
            b2_sb = small.tile([1, OUT], fp32)
            nc.gpsimd.dma_start(b2_sb[:], b2_d[None, :])

            gates_ps = psum.tile([1, R], fp32)

            for g0 in range(0, KT, G):
                gs = min(G, KT - g0)
                wtile = wpool.tile([128, G * R], wdt, tag="wtile")
                nc.sync.dma_start(wtile[:, : gs * R],
                                  wt_d[:, g0 * R:(g0 + gs) * R])
                for t in range(gs):
                    kk = g0 + t
                    for nb in range(NB):
                        nc.tensor.matmul(
                            gates_ps[0:1, nb * 512:(nb + 1) * 512],
                            lhsT=xt_sb[:, kk:kk + 1],
                            rhs=wtile[:, t * R + nb * 512: t * R + (nb + 1) * 512],
                            start=(kk == 0),
                            stop=(kk == KT - 1),
                        )

            # LSTM epilogue on partition 0: gates_ps = [i | f | g | o]
            i_sb = small.tile([1, HS], fp32)
            nc.scalar.activation(i_sb[:], gates_ps[0:1, 0:HS], AF.Sigmoid)
            f_sb = small.tile([1, HS], fp32)
            nc.scalar.activation(f_sb[:], gates_ps[0:1, HS:2 * HS], AF.Sigmoid)
            g_sb = small.tile([1, HS], fp32)
            nc.scalar.activation(g_sb[:], gates_ps[0:1, 2 * HS:3 * HS], AF.Tanh)
            o_sb = small.tile([1, HS], fp32)
            nc.scalar.activation(o_sb[:], gates_ps[0:1, 3 * HS:4 * HS], AF.Sigmoid)

            fc = small.tile([1, HS], fp32)
            nc.vector.tensor_mul(fc[:], f_sb[:], c0_sb[:])
            ig = small.tile([1, HS], fp32)
            nc.vector.tensor_mul(ig[:], i_sb[:], g_sb[:])
            c_sb = small.tile([1, HS], fp32)
            nc.vector.tensor_add(c_sb[:], fc[:], ig[:])
            tch = small.tile([1, HS], fp32)
            nc.scalar.activation(tch[:], c_sb[:], AF.Tanh)
            h_sb = small.tile([1, HS], fp32)
            nc.vector.tensor_mul(h_sb[:], o_sb[:], tch[:])

            if STAGE == "h":
                nc.gpsimd.dma_start(out_d[None, :], h_sb[0:1, :OUT])
            else:
                # partial MLP layer 1: z_part = W1[:, s_k] @ h_k  -> [32].
                # Round-trip h through DRAM to re-tile [1,512] -> [128,4]
                # (partition-major) so the dot runs as 4 K=128 matmuls.
                nc.gpsimd.dma_start(h_d[None, :], h_sb[:])
                hT = small.tile([128, HS // 128], fp32)
                nc.gpsimd.dma_start(
                    hT[:], h_d.ap().rearrange("(t p) -> p t", p=128))

                z_ps = psum.tile([1, HID], fp32)
                for t in range(HS // 128):
                    nc.tensor.matmul(
                        z_ps[:], lhsT=hT[:, t:t + 1], rhs=w1_sb[:, t, :],
                        start=(t == 0), stop=(t == HS // 128 - 1))
                z_sb = small.tile([1, HID], fp32)
                nc.vector.tensor_copy(z_sb[:], z_ps[0:1, :])

                if STAGE == "z":
                    nc.gpsimd.dma_start(out_d[None, :HID], z_sb[:])
                else:
                    nc.gpsimd.dma_start(zp_d[None, :], z_sb[:])
                    nc.gpsimd.collective_compute(
                        "AllReduce",
                        mybir.AluOpType.add,
                        replica_groups=[list(range(NCORES))],
                        ins=[zp_d[:]],
                        outs=[zr_d[:]],
                    )
                    # reload reduced z as [32,1] (partition-per-element)
                    zr_sb = small.tile([HID, 1], fp32)
                    nc.gpsimd.dma_start(zr_sb[:], zr_d[:, None])

                    zb = small.tile([HID, 1], fp32)
                    nc.vector.tensor_add(zb[:], zr_sb[:], b1_sb[:])
                    zrelu = small.tile([HID, 1], fp32)
                    nc.scalar.activation(zrelu[:], zb[:], AF.Relu)

                    out_ps = psum.tile([1, OUT], fp32)
                    nc.tensor.matmul(out_ps[:], lhsT=zrelu[:], rhs=w2_sb[:],
                                     start=True, stop=True)
                    ob = small.tile([1, OUT], fp32)
                    nc.vector.tensor_add(ob[:], out_ps[0:1, :], b2_sb[:])
                    res = small.tile([1, OUT], fp32)
                    nc.scalar.activation(res[:], ob[:], AF.Sigmoid)
                    nc.gpsimd.dma_start(out_d[None, :], res[:])

    nc.compile()
    return nc


def get_nc():
    if "nc" not in _cached:
        _cached["nc"] = build_nc()
    return _cached["nc"]


def shard_inputs(inputs):
    """Slice/transpose/cast the full inputs into per-core input maps."""
    x = np.asarray(inputs["x"], np.float32)
    h0 = np.asarray(inputs["h0"], np.float32)
    c0 = np.asarray(inputs["c0"], np.float32)
    W_ih = np.asarray(inputs["W_ih"], np.float32)
    W_hh = np.asarray(inputs["W_hh"], np.float32)
    b = np.asarray(inputs["b_ih"], np.float32) + np.asarray(inputs["b_hh"], np.float32)
    W1 = np.asarray(inputs["W1"], np.float32)
    b1 = np.asarray(inputs["b1"], np.float32)
    W2 = np.asarray(inputs["W2"], np.float32)
    b2 = np.asarray(inputs["b2"], np.float32)

    xcat = np.zeros(KP, np.float32)
    xcat[:D] = x
    xcat[D:D + H] = h0
    xcat[D + H] = 1.0
    xt = np.ascontiguousarray(xcat.reshape(KT, 128).T).astype(WNP)

    w2t = np.ascontiguousarray(W2.T)

    in_maps = []
    for k in range(NCORES):
        rows = np.concatenate([np.arange(g * H + k * HS, g * H + (k + 1) * HS)
                               for g in range(4)])
        Wfull = np.zeros((R, KP), np.float32)
        Wfull[:, :D] = W_ih[rows]
        Wfull[:, D:D + H] = W_hh[rows]
        Wfull[:, D + H] = b[rows]
        # -> [128(part), KT, R] so each [128, R] K-tile is one contiguous
        #    per-partition chunk: wt[p, t*R + j] = Wfull[j, t*128 + p]
        wt = Wfull.T.reshape(KT, 128, R).transpose(1, 0, 2).astype(WNP)
        # W1 slice, transposed and K-tiled: w1t[p, t*HID + j] = W1[j, k*HS + t*128 + p]
        w1t = (W1[:, k * HS:(k + 1) * HS].T
               .reshape(HS // 128, 128, HID).transpose(1, 0, 2)
               .reshape(128, (HS // 128) * HID))
        in_maps.append({
            "wt": wt.reshape(128, KT * R),
            "xt": xt,
            "c0s": np.ascontiguousarray(c0[k * HS:(k + 1) * HS]),
            "w1t": np.ascontiguousarray(w1t),
            "b1": b1,
            "w2t": w2t,
            "b2": b2,
        })
    return in_maps


def run(inputs, trace=False):
    from concourse.bass_utils import run_bass_kernel_spmd
    nc = get_nc()
    in_maps = shard_inputs(inputs)
    return run_bass_kernel_spmd(nc, in_maps, list(range(NCORES)), trace=trace)


def kernel(**inputs) -> np.ndarray:
    res = run(inputs, trace=False)
    return np.asarray(res.results[0]["out"], np.float32)


# revision 16
# speedup vs baseline: 1.2877x; 1.2877x over previous
"""Bass/Trainium2 kernel for a single LSTM-cell step + tiny MLP head.

Reference computation (all fp32):
    gates = W_ih @ x + b_ih + W_hh @ h0 + b_hh        # [4H], gate order i,f,g,o
    i, f, g, o = sigmoid/sigmoid/tanh/sigmoid splits
    c = f * c0 + i * g ; h = o * tanh(c)              # [H]
    z = relu(W1 @ h + b1)                             # [32]
    out = sigmoid(W2 @ z + b2)                        # [130]

Sharding (8 NeuronCores, tensor-parallel over the hidden dim):
    Core k owns hidden slice s_k = [k*512, (k+1)*512). It gets the four
    512-row blocks of [W_ih | W_hh | b] for its slice, concatenated into
    one [2048, 12293] matrix (bias folded in via a constant-1 appended to
    the x/h0 vector), padded to K=12416 = 97*128 and stored transposed +
    K-tiled so each SBUF tile is a [128(K), 2048(out)] matmul moving
    operand. The big matvec runs on TensorE accumulating into PSUM.
    The LSTM epilogue + partial MLP dot (W1[:, s_k] @ h_k -> [32]) run
    locally; one tiny AllReduce(32 floats) combines the partials and
    every core finishes the replicated MLP head.
"""

import numpy as np
import ml_dtypes

D = 8196
H = 4096
HS = 512           # hidden slice per core
R = 4 * HS         # gate rows per core (2048)
HID = 32
OUT = 130
NCORES = 8
KD = D + H + 1     # x ++ h0 ++ 1.0 (bias column)
KT = 97            # K tiles of 128 (12416 = 97*128 >= 12293)
KP = KT * 128
NB = 4             # PSUM n-blocks of 512 covering R=2048
G = 4              # K-tiles per weight DMA (2 MiB bf16 per dma_start)

WNP = ml_dtypes.bfloat16   # storage dtype for the big weights + x

_cached = {}

# debug bisection: "h" = stop after LSTM h, "z" = stop after local z_part,
# "full" = everything (default)
import os
STAGE = os.environ.get("KERNEL_STAGE", "full")
# matmul free-dim width per instruction (512 = one PSUM bank per matmul)
MMN = int(os.environ.get("KERNEL_MMN", "512"))


def _mybir_wdt(mybir):
    return {
        "bfloat16": mybir.dt.bfloat16,
        "float32": mybir.dt.float32,
        "float8_e4m3fn": mybir.dt.float8e4,
    }[np.dtype(WNP).name]


def build_nc():
    """Build + compile the per-core Bass program (same program on all cores)."""
    import concourse.bass as bass
    import concourse.tile as tile
    from concourse import bacc, mybir

    fp32 = mybir.dt.float32
    wdt = _mybir_wdt(mybir)
    AF = mybir.ActivationFunctionType

    nc = bacc.Bacc("TRN2", target_bir_lowering=False, debug=False,
                   num_devices=NCORES)

    wt_d = nc.dram_tensor("wt", [128, KT * R], wdt, kind="ExternalInput")
    xt_d = nc.dram_tensor("xt", [128, KT], wdt, kind="ExternalInput")
    c0_d = nc.dram_tensor("c0s", [HS], fp32, kind="ExternalInput")
    w1_d = nc.dram_tensor("w1t", [128, (HS // 128) * HID], fp32,
                          kind="ExternalInput")
    b1_d = nc.dram_tensor("b1", [HID], fp32, kind="ExternalInput")
    w2_d = nc.dram_tensor("w2t", [HID, OUT], fp32, kind="ExternalInput")
    b2_d = nc.dram_tensor("b2", [OUT], fp32, kind="ExternalInput")
    out_d = nc.dram_tensor("out", [OUT], fp32, kind="ExternalOutput")

    h_d = nc.dram_tensor("hscratch", [HS], fp32)
    zp_d = nc.dram_tensor("zpart", [HID], fp32)
    zr_d = nc.dram_tensor("zred", [HID], fp32, addr_space="Shared")
    dum_d = nc.dram_tensor("ccdummy", [HID], fp32)
    dumr_d = nc.dram_tensor("ccdummyr", [HID], fp32, addr_space="Shared")

    with tile.TileContext(nc) as tc:
        with (
            tc.tile_pool(name="weights", bufs=3) as wpool,
            tc.tile_pool(name="small", bufs=1) as small,
            tc.tile_pool(name="psum", bufs=1, space="PSUM") as psum,
        ):
            # dummy collective issued first: pays the one-time CC barrier /
            # bootstrap (~50-100us) underneath the weight stream so the real
            # AllReduce later runs warm (~10us instead of ~85us)
            if STAGE == "full":
                nc.gpsimd.collective_compute(
                    "AllReduce",
                    mybir.AluOpType.add,
                    replica_groups=[list(range(NCORES))],
                    ins=[dum_d[:]],
                    outs=[dumr_d[:]],
                )

            # small persistent operands (issued up front, overlap the stream)
            xt_sb = small.tile([128, KT], wdt)
            nc.gpsimd.dma_start(xt_sb[:], xt_d[:])
            c0_sb = small.tile([1, HS], fp32)
            nc.gpsimd.dma_start(c0_sb[:], c0_d[None, :])
            w1_sb = small.tile([128, HS // 128, HID], fp32)
            nc.gpsimd.dma_start(w1_sb[:], w1_d[:])
            b1_sb = small.tile([HID, 1], fp32)
            nc.gpsimd.dma_start(b1_sb[:], b1_d[:, None])
            w2_sb = small.tile([HID, OUT], fp32)
            nc.gpsimd.dma_start(w2_sb[:], w2_d[:])
            b2_sb = small.tile([1, OUT], fp32)
            nc.gpsimd.dma_start(b2_sb[:], b2_d[None, :])

            gates_ps = psum.tile([1, R], fp32)

            # ramp-up: small first groups so the PE starts sooner, then G
            group_sizes = [1, 1, 2]
            rem = KT - sum(group_sizes)
            group_sizes += [G] * (rem // G)
            if rem % G:
                group_sizes.append(rem % G)

            g0 = 0
            for gs in group_sizes:
                wtile = wpool.tile([128, G * R], wdt, tag="wtile")
                nc.sync.dma_start(wtile[:, : gs * R],
                                  wt_d[:, g0 * R:(g0 + gs) * R])
                for t in range(gs):
                    kk = g0 + t
                    for nb in range(R // MMN):
                        nc.tensor.matmul(
                            gates_ps[0:1, nb * MMN:(nb + 1) * MMN],
                            lhsT=xt_sb[:, kk:kk + 1],
                            rhs=wtile[:, t * R + nb * MMN: t * R + (nb + 1) * MMN],
                            start=(kk == 0),
                            stop=(kk == KT - 1),
                        )
                g0 += gs

            # LSTM epilogue on partition 0: gates_ps = [i | f | g | o]
            i_sb = small.tile([1, HS], fp32)
            nc.scalar.activation(i_sb[:], gates_ps[0:1, 0:HS], AF.Sigmoid)
            f_sb = small.tile([1, HS], fp32)
            nc.scalar.activation(f_sb[:], gates_ps[0:1, HS:2 * HS], AF.Sigmoid)
            g_sb = small.tile([1, HS], fp32)
            nc.scalar.activation(g_sb[:], gates_ps[0:1, 2 * HS:3 * HS], AF.Tanh)
            o_sb = small.tile([1, HS], fp32)
            nc.scalar.activation(o_sb[:], gates_ps[0:1, 3 * HS:4 * HS], AF.Sigmoid)

            fc = small.tile([1, HS], fp32)
            nc.vector.tensor_mul(fc[:], f_sb[:], c0_sb[:])
            ig = small.tile([1, HS], fp32)
            nc.vector.tensor_mul(ig[:], i_sb[:], g_sb[:])
            c_sb = small.tile([1, HS], fp32)
            nc.vector.tensor_add(c_sb[:], fc[:], ig[:])
            tch = small.tile([1, HS], fp32)
            nc.scalar.activation(tch[:], c_sb[:], AF.Tanh)
            h_sb = small.tile([1, HS], fp32)
            nc.vector.tensor_mul(h_sb[:], o_sb[:], tch[:])

            if STAGE == "h":
                nc.gpsimd.dma_start(out_d[None, :], h_sb[0:1, :OUT])
            else:
                # partial MLP layer 1: z_part = W1[:, s_k] @ h_k  -> [32].
                # Round-trip h through DRAM to re-tile [1,512] -> [128,4]
                # (partition-major) so the dot runs as 4 K=128 matmuls.
                nc.gpsimd.dma_start(h_d[None, :], h_sb[:])
                hT = small.tile([128, HS // 128], fp32)
                nc.gpsimd.dma_start(
                    hT[:], h_d.ap().rearrange("(t p) -> p t", p=128))

                z_ps = psum.tile([1, HID], fp32)
                for t in range(HS // 128):
                    nc.tensor.matmul(
                        z_ps[:], lhsT=hT[:, t:t + 1], rhs=w1_sb[:, t, :],
                        start=(t == 0), stop=(t == HS // 128 - 1))
                z_sb = small.tile([1, HID], fp32)
                nc.vector.tensor_copy(z_sb[:], z_ps[0:1, :])

                if STAGE == "z":
                    nc.gpsimd.dma_start(out_d[None, :HID], z_sb[:])
                else:
                    nc.gpsimd.dma_start(zp_d[None, :], z_sb[:])
                    nc.gpsimd.collective_compute(
                        "AllReduce",
                        mybir.AluOpType.add,
                        replica_groups=[list(range(NCORES))],
                        ins=[zp_d[:]],
                        outs=[zr_d[:]],
                    )
                    # reload reduced z as [32,1] (partition-per-element)
                    zr_sb = small.tile([HID, 1], fp32)
                    nc.gpsimd.dma_start(zr_sb[:], zr_d[:, None])

                    zb = small.tile([HID, 1], fp32)
                    nc.vector.tensor_add(zb[:], zr_sb[:], b1_sb[:])
                    zrelu = small.tile([HID, 1], fp32)
                    nc.scalar.activation(zrelu[:], zb[:], AF.Relu)

                    out_ps = psum.tile([1, OUT], fp32)
                    nc.tensor.matmul(out_ps[:], lhsT=zrelu[:], rhs=w2_sb[:],
                                     start=True, stop=True)
                    ob = small.tile([1, OUT], fp32)
                    nc.vector.tensor_add(ob[:], out_ps[0:1, :], b2_sb[:])
                    res = small.tile([1, OUT], fp32)
                    nc.scalar.activation(res[:], ob[:], AF.Sigmoid)
                    nc.gpsimd.dma_start(out_d[None, :], res[:])

    nc.compile()
    return nc


def get_nc():
    if "nc" not in _cached:
        _cached["nc"] = build_nc()
    return _cached["nc"]


def shard_inputs(inputs):
    """Slice/transpose/cast the full inputs into per-core input maps."""
    x = np.asarray(inputs["x"], np.float32)
    h0 = np.asarray(inputs["h0"], np.float32)
    c0 = np.asarray(inputs["c0"], np.float32)
    W_ih = np.asarray(inputs["W_ih"], np.float32)
    W_hh = np.asarray(inputs["W_hh"], np.float32)
    b = np.asarray(inputs["b_ih"], np.float32) + np.asarray(inputs["b_hh"], np.float32)
    W1 = np.asarray(inputs["W1"], np.float32)
    b1 = np.asarray(inputs["b1"], np.float32)
    W2 = np.asarray(inputs["W2"], np.float32)
    b2 = np.asarray(inputs["b2"], np.float32)

    xcat = np.zeros(KP, np.float32)
    xcat[:D] = x
    xcat[D:D + H] = h0
    xcat[D + H] = 1.0
    xt = np.ascontiguousarray(xcat.reshape(KT, 128).T).astype(WNP)

    w2t = np.ascontiguousarray(W2.T)

    in_maps = []
    for k in range(NCORES):
        rows = np.concatenate([np.arange(g * H + k * HS, g * H + (k + 1) * HS)
                               for g in range(4)])
        Wfull = np.zeros((R, KP), np.float32)
        Wfull[:, :D] = W_ih[rows]
        Wfull[:, D:D + H] = W_hh[rows]
        Wfull[:, D + H] = b[rows]
        # -> [128(part), KT, R] so each [128, R] K-tile is one contiguous
        #    per-partition chunk: wt[p, t*R + j] = Wfull[j, t*128 + p]
        wt = Wfull.T.reshape(KT, 128, R).transpose(1, 0, 2).astype(WNP)
        # W1 slice, transposed and K-tiled: w1t[p, t*HID + j] = W1[j, k*HS + t*128 + p]
        w1t = (W1[:, k * HS:(k + 1) * HS].T
               .reshape(HS // 128, 128, HID).transpose(1, 0, 2)
               .reshape(128, (HS // 128) * HID))
        in_maps.append({
            "wt": wt.reshape(128, KT * R),
            "xt": xt,
            "c0s": np.ascontiguousarray(c0[k * HS:(k + 1) * HS]),
            "w1t": np.ascontiguousarray(w1t),
            "b1": b1,
            "w2t": w2t,
            "b2": b2,
        })
    return in_maps


def run(inputs, trace=False):
    from concourse.bass_utils import run_bass_kernel_spmd
    nc = get_nc()
    in_maps = shard_inputs(inputs)
    return run_bass_kernel_spmd(nc, in_maps, list(range(NCORES)), trace=trace)


def kernel(**inputs) -> np.ndarray:
    res = run(inputs, trace=False)
    return np.asarray(res.results[0]["out"], np.float32)


# revision 25
# speedup vs baseline: 1.9209x; 1.4917x over previous
"""Bass/Trainium2 kernel for a single LSTM-cell step + tiny MLP head.

Reference computation (all fp32):
    gates = W_ih @ x + b_ih + W_hh @ h0 + b_hh        # [4H], gate order i,f,g,o
    i, f, g, o = sigmoid/sigmoid/tanh/sigmoid splits
    c = f * c0 + i * g ; h = o * tanh(c)              # [H]
    z = relu(W1 @ h + b1)                             # [32]
    out = sigmoid(W2 @ z + b2)                        # [130]

Sharding (8 NeuronCores, tensor-parallel over the hidden dim):
    Core k owns hidden slice s_k = [k*512, (k+1)*512). It gets the four
    512-row blocks of [W_ih | W_hh | b] for its slice, concatenated into
    one [2048, 12293] matrix (bias folded in via a constant-1 appended to
    the x/h0 vector), padded to K=12416 = 97*128 and stored transposed +
    K-tiled so each SBUF tile is a [128(K), 2048(out)] matmul moving
    operand. The big matvec runs on TensorE accumulating into PSUM.
    The LSTM epilogue + partial MLP dot (W1[:, s_k] @ h_k -> [32]) run
    locally; one tiny AllReduce(32 floats) combines the partials and
    every core finishes the replicated MLP head.
"""

import os

import numpy as np
import ml_dtypes

D = 8196
H = 4096
HS = 512           # hidden slice per core
R = 4 * HS         # gate rows per core (2048)
HID = 32
OUT = 130
NCORES = 8
KD = D + H + 1     # x ++ h0 ++ 1.0 (bias column)
KT = 97            # K tiles of 128 (12416 = 97*128 >= 12293)
KP = KT * 128
NB = 4             # PSUM n-blocks of 512 covering R=2048
G = 4              # K-tiles per weight DMA (2 MiB bf16 per dma_start)

_WDTS = {
    "bf16": ml_dtypes.bfloat16,
    "fp8": ml_dtypes.float8_e4m3fn,
    "fp32": np.float32,
}
WNP = _WDTS[os.environ.get("KERNEL_WDT", "bf16")]  # big-weight + x dtype

_cached = {}

# debug bisection: "h" = stop after LSTM h, "z" = stop after local z_part,
# "full" = everything (default)
STAGE = os.environ.get("KERNEL_STAGE", "full")
# matmul free-dim width per instruction (512 = one PSUM bank per matmul)
MMN = int(os.environ.get("KERNEL_MMN", "512"))


def _mybir_wdt(mybir):
    return {
        "bfloat16": mybir.dt.bfloat16,
        "float32": mybir.dt.float32,
        "float8_e4m3fn": mybir.dt.float8e4,
    }[np.dtype(WNP).name if WNP is not np.float32 else "float32"]


def build_nc():
    """Build + compile the per-core Bass program (same program on all cores)."""
    import concourse.bass as bass
    import concourse.tile as tile
    from concourse import bacc, mybir

    fp32 = mybir.dt.float32
    wdt = _mybir_wdt(mybir)
    AF = mybir.ActivationFunctionType

    nc = bacc.Bacc("TRN2", target_bir_lowering=False, debug=False,
                   num_devices=NCORES)

    wt_d = nc.dram_tensor("wt", [128, KT * R], wdt, kind="ExternalInput")
    xt_d = nc.dram_tensor("xt", [128, KT], wdt, kind="ExternalInput")
    c0_d = nc.dram_tensor("c0s", [HS], fp32, kind="ExternalInput")
    w1_d = nc.dram_tensor("w1t", [128, (HS // 128) * HID], fp32,
                          kind="ExternalInput")
    b1_d = nc.dram_tensor("b1", [HID], fp32, kind="ExternalInput")
    w2_d = nc.dram_tensor("w2t", [HID, OUT], fp32, kind="ExternalInput")
    b2_d = nc.dram_tensor("b2", [OUT], fp32, kind="ExternalInput")
    out_d = nc.dram_tensor("out", [OUT], fp32, kind="ExternalOutput")

    h_d = nc.dram_tensor("hscratch", [HS], fp32)
    zp_d = nc.dram_tensor("zpart", [HID], fp32)
    zr_d = nc.dram_tensor("zred", [HID], fp32, addr_space="Shared")
    dum_d = nc.dram_tensor("ccdummy", [HID], fp32)
    dumr_d = nc.dram_tensor("ccdummyr", [HID], fp32, addr_space="Shared")

    with tile.TileContext(nc) as tc:
        with (
            tc.tile_pool(name="weights", bufs=3) as wpool,
            tc.tile_pool(name="small", bufs=1) as small,
            tc.tile_pool(name="psum", bufs=1, space="PSUM") as psum,
        ):
            # dummy collective issued first: pays the one-time CC barrier /
            # bootstrap (~50-100us) underneath the weight stream so the real
            # AllReduce later runs warm (~10us instead of ~85us)
            if STAGE == "full":
                zt = small.tile([1, HID], fp32)
                nc.gpsimd.memset(zt[:], 0.0)
                nc.gpsimd.dma_start(dum_d[None, :], zt[:])
                nc.gpsimd.collective_compute(
                    "AllReduce",
                    mybir.AluOpType.add,
                    replica_groups=[list(range(NCORES))],
                    ins=[dum_d[:]],
                    outs=[dumr_d[:]],
                )

            # small persistent operands (issued up front, overlap the stream)
            xt_sb = small.tile([128, KT], wdt)
            nc.gpsimd.dma_start(xt_sb[:], xt_d[:])
            c0_sb = small.tile([1, HS], fp32)
            nc.gpsimd.dma_start(c0_sb[:], c0_d[None, :])
            w1_sb = small.tile([128, HS // 128, HID], fp32)
            nc.gpsimd.dma_start(w1_sb[:], w1_d[:])
            b1_sb = small.tile([HID, 1], fp32)
            nc.gpsimd.dma_start(b1_sb[:], b1_d[:, None])
            w2_sb = small.tile([HID, OUT], fp32)
            nc.gpsimd.dma_start(w2_sb[:], w2_d[:])
            b2_sb = small.tile([1, OUT], fp32)
            nc.gpsimd.dma_start(b2_sb[:], b2_d[None, :])

            gates_ps = psum.tile([1, R], fp32)

            # ramp-up: small first groups so the PE starts sooner, then G
            group_sizes = [1, 1, 2]
            rem = KT - sum(group_sizes)
            group_sizes += [G] * (rem // G)
            if rem % G:
                group_sizes.append(rem % G)

            g0 = 0
            for gs in group_sizes:
                wtile = wpool.tile([128, G * R], wdt, tag="wtile")
                nc.sync.dma_start(wtile[:, : gs * R],
                                  wt_d[:, g0 * R:(g0 + gs) * R])
                for t in range(gs):
                    kk = g0 + t
                    for nb in range(R // MMN):
                        nc.tensor.matmul(
                            gates_ps[0:1, nb * MMN:(nb + 1) * MMN],
                            lhsT=xt_sb[:, kk:kk + 1],
                            rhs=wtile[:, t * R + nb * MMN: t * R + (nb + 1) * MMN],
                            start=(kk == 0),
                            stop=(kk == KT - 1),
                        )
                g0 += gs

            # LSTM epilogue on partition 0: gates_ps = [i | f | g | o]
            i_sb = small.tile([1, HS], fp32)
            nc.scalar.activation(i_sb[:], gates_ps[0:1, 0:HS], AF.Sigmoid)
            f_sb = small.tile([1, HS], fp32)
            nc.scalar.activation(f_sb[:], gates_ps[0:1, HS:2 * HS], AF.Sigmoid)
            g_sb = small.tile([1, HS], fp32)
            nc.scalar.activation(g_sb[:], gates_ps[0:1, 2 * HS:3 * HS], AF.Tanh)
            o_sb = small.tile([1, HS], fp32)
            nc.scalar.activation(o_sb[:], gates_ps[0:1, 3 * HS:4 * HS], AF.Sigmoid)

            fc = small.tile([1, HS], fp32)
            nc.vector.tensor_mul(fc[:], f_sb[:], c0_sb[:])
            ig = small.tile([1, HS], fp32)
            nc.vector.tensor_mul(ig[:], i_sb[:], g_sb[:])
            c_sb = small.tile([1, HS], fp32)
            nc.vector.tensor_add(c_sb[:], fc[:], ig[:])
            tch = small.tile([1, HS], fp32)
            nc.scalar.activation(tch[:], c_sb[:], AF.Tanh)
            h_sb = small.tile([1, HS], fp32)
            nc.vector.tensor_mul(h_sb[:], o_sb[:], tch[:])

            if STAGE == "h":
                nc.gpsimd.dma_start(out_d[None, :], h_sb[0:1, :OUT])
            else:
                # partial MLP layer 1: z_part = W1[:, s_k] @ h_k  -> [32].
                # Round-trip h through DRAM to re-tile [1,512] -> [128,4]
                # (partition-major) so the dot runs as 4 K=128 matmuls.
                nc.gpsimd.dma_start(h_d[None, :], h_sb[:])
                hT = small.tile([128, HS // 128], fp32)
                nc.gpsimd.dma_start(
                    hT[:], h_d.ap().rearrange("(t p) -> p t", p=128))

                z_ps = psum.tile([1, HID], fp32)
                for t in range(HS // 128):
                    nc.tensor.matmul(
                        z_ps[:], lhsT=hT[:, t:t + 1], rhs=w1_sb[:, t, :],
                        start=(t == 0), stop=(t == HS // 128 - 1))
                z_sb = small.tile([1, HID], fp32)
                nc.vector.tensor_copy(z_sb[:], z_ps[0:1, :])

                if STAGE == "z":
                    nc.gpsimd.dma_start(out_d[None, :HID], z_sb[:])
                else:
                    nc.gpsimd.dma_start(zp_d[None, :], z_sb[:])
                    nc.gpsimd.collective_compute(
                        "AllReduce",
                        mybir.AluOpType.add,
                        replica_groups=[list(range(NCORES))],
                        ins=[zp_d[:]],
                        outs=[zr_d[:]],
                    )
                    # reload reduced z as [32,1] (partition-per-element)
                    zr_sb = small.tile([HID, 1], fp32)
                    nc.gpsimd.dma_start(zr_sb[:], zr_d[:, None])

                    zb = small.tile([HID, 1], fp32)
                    nc.vector.tensor_add(zb[:], zr_sb[:], b1_sb[:])
                    zrelu = small.tile([HID, 1], fp32)
                    nc.scalar.activation(zrelu[:], zb[:], AF.Relu)

                    out_ps = psum.tile([1, OUT], fp32)
                    nc.tensor.matmul(out_ps[:], lhsT=zrelu[:], rhs=w2_sb[:],
                                     start=True, stop=True)
                    ob = small.tile([1, OUT], fp32)
                    nc.vector.tensor_add(ob[:], out_ps[0:1, :], b2_sb[:])
                    res = small.tile([1, OUT], fp32)
                    nc.scalar.activation(res[:], ob[:], AF.Sigmoid)
                    nc.gpsimd.dma_start(out_d[None, :], res[:])

    nc.compile()
    return nc


def get_nc():
    if "nc" not in _cached:
        _cached["nc"] = build_nc()
    return _cached["nc"]


def shard_inputs(inputs):
    """Slice/transpose/cast the full inputs into per-core input maps."""
    x = np.asarray(inputs["x"], np.float32)
    h0 = np.asarray(inputs["h0"], np.float32)
    c0 = np.asarray(inputs["c0"], np.float32)
    W_ih = np.asarray(inputs["W_ih"], np.float32)
    W_hh = np.asarray(inputs["W_hh"], np.float32)
    b = np.asarray(inputs["b_ih"], np.float32) + np.asarray(inputs["b_hh"], np.float32)
    W1 = np.asarray(inputs["W1"], np.float32)
    b1 = np.asarray(inputs["b1"], np.float32)
    W2 = np.asarray(inputs["W2"], np.float32)
    b2 = np.asarray(inputs["b2"], np.float32)

    xcat = np.zeros(KP, np.float32)
    xcat[:D] = x
    xcat[D:D + H] = h0
    xcat[D + H] = 1.0
    xt = np.ascontiguousarray(xcat.reshape(KT, 128).T).astype(WNP)

    w2t = np.ascontiguousarray(W2.T)

    in_maps = []
    for k in range(NCORES):
        rows = np.concatenate([np.arange(g * H + k * HS, g * H + (k + 1) * HS)
                               for g in range(4)])
        Wfull = np.zeros((R, KP), np.float32)
        Wfull[:, :D] = W_ih[rows]
        Wfull[:, D:D + H] = W_hh[rows]
        Wfull[:, D + H] = b[rows]
        # -> [128(part), KT, R] so each [128, R] K-tile is one contiguous
        #    per-partition chunk: wt[p, t*R + j] = Wfull[j, t*128 + p]
        wt = Wfull.T.reshape(KT, 128, R).transpose(1, 0, 2).astype(WNP)
        # W1 slice, transposed and K-tiled: w1t[p, t*HID + j] = W1[j, k*HS + t*128 + p]
        w1t = (W1[:, k * HS:(k + 1) * HS].T
               .reshape(HS // 128, 128, HID).transpose(1, 0, 2)
               .reshape(128, (HS // 128) * HID))
        in_maps.append({
            "wt": wt.reshape(128, KT * R),
            "xt": xt,
            "c0s": np.ascontiguousarray(c0[k * HS:(k + 1) * HS]),
            "w1t": np.ascontiguousarray(w1t),
            "b1": b1,
            "w2t": w2t,
            "b2": b2,
        })
    return in_maps


def run(inputs, trace=False):
    from concourse.bass_utils import run_bass_kernel_spmd
    nc = get_nc()
    in_maps = shard_inputs(inputs)
    return run_bass_kernel_spmd(nc, in_maps, list(range(NCORES)), trace=trace)


def kernel(**inputs) -> np.ndarray:
    res = run(inputs, trace=False)
    return np.asarray(res.results[0]["out"], np.float32)
